# revision 8
# baseline (speedup 1.0000x reference)
"""DeepFM (nn_DeepFM_66331474919973) Trainium2 Bass kernel — v2.

Architecture (all gathers via [128,1]-offset indirect DMA, the only
bulk-gather primitive verified bit-correct on this runtime; dma_gather /
multi-column indirect both fault):

  - 8 cores x 2048 batch rows, data parallel; tables replicated, packed
    bf16 into one mega-table [user|item|brand|text|zero-row] x 66
    (64 emb + 1 lin + 1 pad).
  - Host sorts batch rows by text-token count and deals 128-row groups
    round-robin to (core, sub-block) so each sub-block's token-column
    count is near-uniform -> minimal gather padding. Pad offsets hit a
    zero row.
  - user/item/brand/text rows fetched with one indirect_dma_start per
    128 rows (bf16, 132B/row). Text pooling = in-place halving adds on
    DVE over the gathered [128, cnt, 66] tile.
  - cat field (vocab 500) never touches DMA: table lives in SBUF; per
    block, 8 slots x 4 vocab-chunks of one-hot [128,512] masks (DVE
    is_equal on f16 values) are matmul'd against the table chunk,
    PSUM-accumulating v_cat^T (+ its lin row) directly in transposed
    layout.
  - sales field = rank-2 outer-product matmul (projw/projb x salesT/1).
  - FM + MLP run in the transposed domain [384 x 512] per block:
    sum-vector via a halves-selector matmul, squares on ScalarE,
    row-sums as +-0.5-weighted ones-matmuls, first-order terms +
    deep output all accumulate into a single [1,512] PSUM chain,
    finished by one fused Sigmoid (bias = b3 + sales_lin_b).
"""
import sys

sys.path.insert(0, "/opt/trn_rl_repo")

import numpy as np
import ml_dtypes
from contextlib import ExitStack

import concourse.bass as bass
import concourse.tile as tile
from concourse import bacc, mybir
from concourse.bass_utils import run_bass_kernel_spmd
from concourse.masks import make_identity

# ---- problem constants ----
B, K = 16384, 64
NU, NI, BV, CV, TB = 500000, 500000, 1000, 500, 100000
LC, LT = 8, 64
H1, H2 = 128, 64
NCORES = 8
BC = B // NCORES       # 2048 rows per core
NSUB = BC // 128       # 16 sub-blocks per core
NBLK = BC // 512       # 4 blocks per core
SPB = 4                # sub-blocks per block
D = 66                 # packed row: 64 emb + 1 lin + 1 pad

# mega-table row offsets
OU, OI, OB, OT = 0, NU, NU + NI, NU + NI + BV
OZ = OT + TB           # zero row
NMEGA = OZ + 1

# con (f32 consts) columns
C_B1, C_B2, C_B3S, C_SLW = 0, 1, 2, 3
NCON = 4

BF16 = ml_dtypes.bfloat16
F16 = np.float16

_BUILD_CACHE = {}


def build_program(cnts):
    """cnts: tuple of NSUB ints — padded text column count per sub-block
    (shared across cores)."""
    tot = sum(cnts)
    base = [0] * NSUB
    for s in range(1, NSUB):
        base[s] = base[s - 1] + cnts[s - 1]

    nc = bacc.Bacc(
        "TRN2", target_bir_lowering=False, debug=False,
        enable_asserts=False, num_devices=NCORES,
    )
    f32, i32 = mybir.dt.float32, mybir.dt.int32
    bf16, f16 = mybir.dt.bfloat16, mybir.dt.float16
    AF = mybir.ActivationFunctionType
    OP = mybir.AluOpType

    mega_d = nc.dram_tensor("mega", [NMEGA, D], bf16, kind="ExternalInput")
    toff_d = nc.dram_tensor("toff", [128, tot], i32, kind="ExternalInput")
    uibo_d = nc.dram_tensor("uibo", [128, 48], i32, kind="ExternalInput")
    cattab_d = nc.dram_tensor("cattab", [128, 4 * D], bf16, kind="ExternalInput")
    catval_d = nc.dram_tensor("catval", [128, LC * NBLK * 512], f16,
                              kind="ExternalInput")
    iotac_d = nc.dram_tensor("iotac", [128, 4], f16, kind="ExternalInput")
    rct_d = nc.dram_tensor("rct", [128, NBLK * 512], bf16, kind="ExternalInput")
    scal_d = nc.dram_tensor("scal", [128, NSUB * 2], f32, kind="ExternalInput")
    sal2_d = nc.dram_tensor("sal2", [2, NBLK * 512], f32, kind="ExternalInput")
    con_d = nc.dram_tensor("con", [128, NCON], f32, kind="ExternalInput")
    cbf_d = nc.dram_tensor("cbf", [128, 4], bf16, kind="ExternalInput")
    w1_d = nc.dram_tensor("w1", [128, 3 * H1], bf16, kind="ExternalInput")
    w2_d = nc.dram_tensor("w2", [128, H2], bf16, kind="ExternalInput")
    sproj_d = nc.dram_tensor("sproj", [2, 128], f32, kind="ExternalInput")
    out_d = nc.dram_tensor("out", [1, BC], f32, kind="ExternalOutput")

    with tile.TileContext(nc) as tc, ExitStack() as ctx:
        cpool = ctx.enter_context(tc.tile_pool(name="const", bufs=1))
        gpool = ctx.enter_context(tc.tile_pool(name="gath", bufs=2))
        xpool = ctx.enter_context(tc.tile_pool(name="xt", bufs=2))
        opool = ctx.enter_context(tc.tile_pool(name="oh", bufs=4))
        ppool = ctx.enter_context(tc.tile_pool(name="ps", bufs=1, space="PSUM"))

        # ---------- consts ----------
        con = cpool.tile([128, NCON], f32)
        nc.sync.dma_start(con[:], con_d.ap())
        cbf = cpool.tile([128, 4], bf16)
        nc.sync.dma_start(cbf[:], cbf_d.ap())
        w1 = cpool.tile([128, 3, H1], bf16)
        nc.sync.dma_start(w1[:], w1_d.ap())
        w2 = cpool.tile([128, H2], bf16)
        nc.sync.dma_start(w2[:], w2_d.ap())
        sproj = cpool.tile([2, 128], f32)
        nc.sync.dma_start(sproj[:], sproj_d.ap())
        sal2 = cpool.tile([2, NBLK, 512], f32)
        nc.sync.dma_start(sal2[:], sal2_d.ap())
        cattab = cpool.tile([128, 4, D], bf16)
        nc.sync.dma_start(cattab[:], cattab_d.ap())
        catval = cpool.tile([128, LC * NBLK, 512], f16)
        nc.sync.dma_start(catval[:], catval_d.ap())
        iotac = cpool.tile([128, 4], f16)
        nc.sync.dma_start(iotac[:], iotac_d.ap())
        rct = cpool.tile([128, NBLK, 512], bf16)
        nc.sync.dma_start(rct[:], rct_d.ap())
        scal = cpool.tile([128, NSUB, 2], f32)
        nc.sync.dma_start(scal[:], scal_d.ap())
        toff = cpool.tile([128, tot], i32)
        nc.sync.dma_start(toff[:], toff_d.ap())
        uibo = cpool.tile([128, 48], i32)
        nc.sync.dma_start(uibo[:], uibo_d.ap())
        identb = cpool.tile([128, 128], bf16)
        make_identity(nc, identb[:])
        S = cpool.tile([128, H2], bf16)
        nc.vector.tensor_add(S[:], identb[:, 0:64], identb[:, 64:128])

        # ---------- per-block ----------
        for blk in range(NBLK):
            # --- gathers (issue first so they pipeline ahead) ---
            gub = gpool.tile([128, 12, D], bf16, tag="gub")
            for s4 in range(SPB):
                s = SPB * blk + s4
                nc.gpsimd.indirect_dma_start(
                    out=gub[:, 2 * s4, :], out_offset=None, in_=mega_d.ap(),
                    in_offset=bass.IndirectOffsetOnAxis(
                        ap=uibo[:, s : s + 1], axis=0),
                )
                nc.gpsimd.indirect_dma_start(
                    out=gub[:, 2 * s4 + 1, :], out_offset=None, in_=mega_d.ap(),
                    in_offset=bass.IndirectOffsetOnAxis(
                        ap=uibo[:, 16 + s : 17 + s], axis=0),
                )
                nc.gpsimd.indirect_dma_start(
                    out=gub[:, 8 + s4, :], out_offset=None, in_=mega_d.ap(),
                    in_offset=bass.IndirectOffsetOnAxis(
                        ap=uibo[:, 32 + s : 33 + s], axis=0),
                )
            tts = []
            for s4 in range(SPB):
                s = SPB * blk + s4
                cnt = cnts[s]
                tt = gpool.tile([128, cnt, D], bf16, tag=f"tt{s4}")
                for j in range(cnt):
                    col = base[s] + j
                    nc.gpsimd.indirect_dma_start(
                        out=tt[:, j, :], out_offset=None, in_=mega_d.ap(),
                        in_offset=bass.IndirectOffsetOnAxis(
                            ap=toff[:, col : col + 1], axis=0),
                    )
                tts.append(tt)

            # --- cat one-hot chain -> vcps [D, 512] (v_cat^T, lin at row 64)
            vcps = ppool.tile([D, 512], f32, tag="vcps")
            k = 0
            for ch in range(4):
                for l in range(LC):
                    oh = opool.tile([128, 512], bf16, tag="oh")
                    nc.vector.tensor_tensor(
                        oh[:], catval[:, l * NBLK + blk, :],
                        iotac[:, ch : ch + 1].to_broadcast([128, 512]),
                        OP.is_equal,
                    )
                    nc.tensor.matmul(
                        vcps[:], cattab[:, ch, :], oh[:],
                        start=(k == 0), stop=(k == 4 * LC - 1),
                    )
                    k += 1
            catsb = xpool.tile([D, 512], bf16, tag="catsb")
            nc.scalar.copy(catsb[:], vcps[:])

            # --- sales outer product -> vs_ps rows 64:128 = v_sales^T
            vs_ps = ppool.tile([128, 512], f32, tag="vsps")
            nc.tensor.matmul(
                vs_ps[:], sproj[:], sal2[:, blk, :], start=True, stop=True,
            )

            # --- contiguous (u|i) pairs for the transposes ---
            ffui = xpool.tile([128, SPB, 2, K], bf16, tag="ffui")
            nc.vector.tensor_copy(
                ffui[:].rearrange("p a b k -> p (a b) k"), gub[:, 0:8, 0:K]
            )

            # --- text trees (halving adds) + ffbt (brand|text row-major) ---
            ffbt = xpool.tile([128, SPB, 2, K], bf16, tag="ffbt")
            nc.vector.tensor_copy(ffbt[:, :, 0, :], gub[:, 8:12, 0:K])
            for s4 in range(SPB):
                s = SPB * blk + s4
                tt = tts[s4]
                c = cnts[s]
                while c > 1:
                    h = c // 2
                    nc.vector.tensor_add(
                        tt[:, 0:h, :], tt[:, 0:h, :], tt[:, c - h : c, :],
                    )
                    c = c - h
                nc.vector.tensor_scalar(
                    ffbt[:, s4, 1, :], tt[:, 0, 0:K],
                    scal[:, s, 1:2], None, OP.mult,
                )

            # --- transposes: xt0 = (u|i)^T, xt1 = (b|t)^T ---
            pt0 = ppool.tile([128, 512], bf16, tag="pt0")
            pt1 = ppool.tile([128, 512], bf16, tag="pt1")
            for s4 in range(SPB):
                nc.tensor.transpose(
                    pt0[:, 128 * s4 : 128 * (s4 + 1)],
                    ffui[:, s4, :, :].rearrange("p a k -> p (a k)"), identb[:],
                )
                nc.tensor.transpose(
                    pt1[:, 128 * s4 : 128 * (s4 + 1)],
                    ffbt[:, s4, :, :].rearrange("p a k -> p (a k)"), identb[:],
                )
            xt0 = xpool.tile([128, 512], bf16, tag="xt0")
            nc.scalar.copy(xt0[:], pt0[:])
            xt1 = xpool.tile([128, 512], bf16, tag="xt1")
            nc.scalar.copy(xt1[:], pt1[:])
            xt2 = xpool.tile([128, 512], bf16, tag="xt2")
            nc.vector.tensor_tensor(
                xt2[0:64, :], catsb[0:64, :], rct[0:64, blk, :], OP.mult,
            )
            nc.scalar.copy(xt2[64:128, :], vs_ps[64:128, :])

            # --- first-order row-major part -> rb [128, SPB] bf16 ---
            rb = xpool.tile([128, SPB], bf16, tag="rb")
            tmp = xpool.tile([128, SPB], f32, tag="tmp")
            nc.vector.tensor_scalar(
                tmp[:], scal[:, SPB * blk : SPB * (blk + 1), 0],
                con[:, C_SLW : C_SLW + 1], None, OP.mult,
            )
            for s4 in range(SPB):
                nc.vector.tensor_add(
                    rb[:, s4 : s4 + 1], gub[:, 2 * s4, 64:65],
                    gub[:, 2 * s4 + 1, 64:65],
                )
                nc.vector.tensor_add(
                    rb[:, s4 : s4 + 1], rb[:, s4 : s4 + 1],
                    gub[:, 8 + s4, 64:65],
                )
                nc.vector.tensor_add(
                    rb[:, s4 : s4 + 1], rb[:, s4 : s4 + 1],
                    tts[s4][:, 0, 64:65],
                )
                nc.vector.tensor_add(
                    rb[:, s4 : s4 + 1], rb[:, s4 : s4 + 1],
                    tmp[:, s4 : s4 + 1],
                )

            # --- logits accumulation chain in fo_ps [1, 512] ---
            fo_ps = ppool.tile([1, 512], f32, tag="fops")
            # cat lin row (partition 64 of catsb) -> [1, 512]
            nc.tensor.matmul(
                fo_ps[:], identb[0:D, 64:65], catsb[:], start=True, stop=False,
            )
            for s4 in range(SPB):
                nc.tensor.matmul(
                    fo_ps[:, 128 * s4 : 128 * (s4 + 1)],
                    rb[:, s4 : s4 + 1], identb[:], start=False, stop=False,
                )

            # --- FM second order ---
            sv_ps = ppool.tile([64, 512], f32, tag="svps")
            for c, xt in enumerate((xt0, xt1, xt2)):
                nc.tensor.matmul(
                    sv_ps[:], S[:], xt[:], start=(c == 0), stop=(c == 2),
                )
            sv2 = xpool.tile([64, 512], bf16, tag="sv2")
            nc.scalar.activation(sv2[:], sv_ps[:], AF.Square)
            nc.tensor.matmul(
                fo_ps[:], cbf[0:64, 1:2], sv2[:], start=False, stop=False,
            )
            for c, xt in enumerate((xt0, xt1, xt2)):
                sq = xpool.tile([128, 512], bf16, tag=f"sq{c}")
                nc.scalar.activation(sq[:], xt[:], AF.Square)
                nc.tensor.matmul(
                    fo_ps[:], cbf[:, 2:3], sq[:], start=False, stop=False,
                )

            # --- deep MLP ---
            ph1 = ppool.tile([128, 512], f32, tag="ph1")
            for c, xt in enumerate((xt0, xt1, xt2)):
                nc.tensor.matmul(
                    ph1[:], w1[:, c, :], xt[:], start=(c == 0), stop=(c == 2),
                )
            h1 = xpool.tile([128, 512], bf16, tag="h1")
            nc.scalar.activation(
                h1[:], ph1[:], AF.Relu, bias=con[:, C_B1 : C_B1 + 1],
            )
            ph2 = ppool.tile([64, 512], f32, tag="ph2")
            nc.tensor.matmul(ph2[:], w2[:], h1[:], start=True, stop=True)
            h2 = xpool.tile([64, 512], bf16, tag="h2")
            nc.scalar.activation(
                h2[:], ph2[:], AF.Relu, bias=con[0:64, C_B2 : C_B2 + 1],
            )
            nc.tensor.matmul(
                fo_ps[:], cbf[0:64, 0:1], h2[:], start=False, stop=True,
            )

            # --- sigmoid + store ---
            sig = xpool.tile([1, 512], f32, tag="sig")
            nc.scalar.activation(
                sig[:], fo_ps[:], AF.Sigmoid,
                bias=con[0:1, C_B3S : C_B3S + 1],
            )
            nc.sync.dma_start(out_d.ap()[0:1, 512 * blk : 512 * (blk + 1)], sig[:])

    nc.compile()
    return nc


def _prep(inputs):
    f32 = np.float32

    def bf(x):
        return np.asarray(x, f32).astype(BF16)

    # ---- mega table ----
    mega = np.zeros((NMEGA, D), BF16)
    mega[OU : OU + NU, 0:K] = bf(inputs["user_emb_w"])
    mega[OU : OU + NU, K] = bf(inputs["user_lin_w"])[:, 0]
    mega[OI : OI + NI, 0:K] = bf(inputs["item_emb_w"])
    mega[OI : OI + NI, K] = bf(inputs["item_lin_w"])[:, 0]
    mega[OB : OB + BV, 0:K] = bf(inputs["brand_emb_w"])
    mega[OB : OB + BV, K] = bf(inputs["brand_lin_w"])[:, 0]
    mega[OT : OT + TB, 0:K] = bf(inputs["text_emb_w"])
    mega[OT : OT + TB, K] = bf(inputs["text_lin_w"])[:, 0]

    user = np.asarray(inputs["user"]).astype(np.int64)
    item = np.asarray(inputs["item"]).astype(np.int64)
    brand = np.asarray(inputs["brand_idx"]).astype(np.int64)
    cat_idx = np.asarray(inputs["cat_idx"]).astype(np.int64)
    cat_mask = np.asarray(inputs["cat_mask"]).astype(bool)
    text_idx = np.asarray(inputs["text_idx"]).astype(np.int64)
    text_mask = np.asarray(inputs["text_mask"]).astype(bool)
    sales = np.asarray(inputs["sales_rank"], f32)[:, 0]

    counts = text_mask.sum(-1).astype(np.int64)           # [B]
    order = np.argsort(-counts, kind="stable")
    # rows_by_core[c, s, p] = original row id
    rows = np.empty((NCORES, NSUB, 128), np.int64)
    for s in range(NSUB):
        for c in range(NCORES):
            g = NCORES * s + c
            rows[c, s] = order[128 * g : 128 * (g + 1)]

    cnts = tuple(
        int(max(1, counts[rows[:, s, :]].max())) for s in range(NSUB)
    )
    tot = sum(cnts)

    cat_eff = np.where(cat_mask, cat_idx, CV).astype(np.int64)
    recip_c = (1.0 / np.maximum(cat_mask.sum(-1), 1)).astype(f32)
    recip_t = (1.0 / np.maximum(counts, 1)).astype(f32)

    # ---- shared consts ----
    cattab = np.zeros((128, 4, D), BF16)
    catpad = np.zeros((512, D), f32)
    catpad[:CV, 0:K] = np.asarray(inputs["cat_emb_w"], f32)
    catpad[:CV, K] = np.asarray(inputs["cat_lin_w"], f32)[:, 0]
    for ch in range(4):
        cattab[:, ch, :] = catpad[128 * ch : 128 * (ch + 1)].astype(BF16)
    cattab = cattab.reshape(128, 4 * D)

    iotac = (np.arange(128, dtype=f32)[:, None]
             + 128.0 * np.arange(4, dtype=f32)[None, :]).astype(F16)

    W1 = np.asarray(inputs["W1"], f32)            # [384, 128]
    w1t = np.empty((128, 3, H1), BF16)
    w1t[:, 0] = bf(W1[0:128])
    w1t[:, 1] = bf(np.concatenate([W1[128:192], W1[256:320]]))
    w1t[:, 2] = bf(np.concatenate([W1[192:256], W1[320:384]]))
    w1t = w1t.reshape(128, 3 * H1)
    w2t = bf(np.asarray(inputs["W2"], f32))       # [128, 64]

    con = np.zeros((128, NCON), f32)
    con[:, C_B1] = np.asarray(inputs["b1"], f32)
    con[0:64, C_B2] = np.asarray(inputs["b2"], f32)
    con[:, C_B3S] = float(np.asarray(inputs["b3"], f32)[0]
                          + np.asarray(inputs["sales_lin_b"], f32)[0])
    con[:, C_SLW] = float(np.asarray(inputs["sales_lin_w"], f32)[0, 0])

    cbf = np.zeros((128, 4), BF16)
    cbf[0:64, 0] = bf(np.asarray(inputs["W3"], f32)[:, 0])
    cbf[0:64, 1] = BF16(0.5)
    cbf[:, 2] = BF16(-0.5)

    sproj = np.zeros((2, 128), f32)
    sproj[0, 64:128] = np.asarray(inputs["sales_proj_w"], f32)[0]
    sproj[1, 64:128] = np.asarray(inputs["sales_proj_b"], f32)

    # ---- per-core tensors ----
    in_maps = []
    for c in range(NCORES):
        r = rows[c]                                # [NSUB, 128]
        m = text_mask[r]                           # [NSUB, 128, LT]
        ti = text_idx[r]
        ordt = np.argsort(~m, axis=-1, kind="stable")
        tok = np.take_along_axis(ti, ordt, axis=-1)
        cnt_row = counts[r]                        # [NSUB, 128]

        toff = np.empty((128, tot), np.int32)
        pos = 0
        for s in range(NSUB):
            cs = cnts[s]          # cs <= LT always (cnt is a max of counts)
            j = np.arange(cs)[None, :]
            valid = j < cnt_row[s][:, None]
            off = np.where(valid, OT + tok[s][:, :cs], OZ)
            toff[:, pos : pos + cs] = off.astype(np.int32)
            pos += cs

        uibo = np.empty((128, 48), np.int32)
        for s in range(NSUB):
            uibo[:, s] = (OU + user[r[s]]).astype(np.int32)
            uibo[:, 16 + s] = (OI + item[r[s]]).astype(np.int32)
            uibo[:, 32 + s] = (OB + brand[r[s]]).astype(np.int32)

        cv = np.empty((LC * NBLK, 512), F16)
        for l in range(LC):
            for blk in range(NBLK):
                vals = cat_eff[r[SPB * blk : SPB * (blk + 1)], l]  # [4,128]
                cv[l * NBLK + blk] = vals.reshape(512).astype(F16)
        catval = np.broadcast_to(cv[None, :, :], (128, LC * NBLK, 512))
        catval = np.ascontiguousarray(catval).reshape(128, LC * NBLK * 512)

        rc = np.empty((NBLK, 512), f32)
        for blk in range(NBLK):
            rc[blk] = recip_c[r[SPB * blk : SPB * (blk + 1)]].reshape(512)
        rct = np.broadcast_to(rc.astype(BF16)[None], (128, NBLK, 512))
        rct = np.ascontiguousarray(rct).reshape(128, NBLK * 512)

        scal = np.empty((128, NSUB, 2), f32)
        for s in range(NSUB):
            scal[:, s, 0] = sales[r[s]]
            scal[:, s, 1] = recip_t[r[s]]
        scal = scal.reshape(128, NSUB * 2)

        sal2 = np.empty((2, NBLK, 512), f32)
        for blk in range(NBLK):
            sal2[0, blk] = sales[r[SPB * blk : SPB * (blk + 1)]].reshape(512)
        sal2[1] = 1.0
        sal2 = sal2.reshape(2, NBLK * 512)

        in_maps.append({
            "mega": mega,
            "toff": toff,
            "uibo": uibo,
            "cattab": cattab,
            "catval": catval,
            "iotac": iotac,
            "rct": rct,
            "scal": scal,
            "sal2": sal2,
            "con": con,
            "cbf": cbf,
            "w1": w1t,
            "w2": w2t,
            "sproj": sproj,
        })
    return cnts, in_maps, rows


LAST_RESULTS = None


def kernel(**inputs):
    global LAST_RESULTS
    import os

    cnts, in_maps, rows = _prep(inputs)
    if cnts not in _BUILD_CACHE:
        _BUILD_CACHE[cnts] = build_program(cnts)
    nc = _BUILD_CACHE[cnts]

    ncores = int(os.environ.get("KER_CORES", str(NCORES)))
    trace = bool(int(os.environ.get("KER_TRACE", "0")))
    try:
        res = run_bass_kernel_spmd(
            nc, in_maps[:ncores], list(range(ncores)), trace=trace
        )
        LAST_RESULTS = res
        outs = [res.results[c]["out"][0] for c in range(ncores)]
    except Exception as e:
        sys.stderr.write(f"kernel: device run failed ({e!r}); CoreSim fallback\n")
        from concourse.bass_interp import CoreSim

        outs = []
        for c in range(ncores):
            sim = CoreSim(nc)
            for k2, v2 in in_maps[c].items():
                sim.tensor(k2)[:] = v2
            sim.simulate()
            outs.append(np.array(sim.tensor("out")[0]))

    full = np.zeros(B, np.float32)
    for c in range(ncores):
        for s in range(NSUB):
            full[rows[c, s]] = outs[c][128 * s : 128 * (s + 1)]
    return full
